# revision 4
# baseline (speedup 1.0000x reference)
"""Distributed causal multi-head attention for 8 TRN2 NeuronCores (v3).

Problem: B=2, T=2048, D=1024, H=16 heads (hd=64), f32 in/out.

Sharding: core i handles batch b=i//4 and head-group g=i%4 (4 heads).
Wq/Wk/Wv column-sharded ([1024, 256] per core), Wo row-sharded
([256, 1024] per core).  Each core computes a partial output projection
for its 4 heads over the full sequence; the host sums the 4 partials
per batch (the unshard step replaces the all-reduce).  Weights and
activations are pre-cast to bf16 on the host; x is laid out transposed
(xT = x^T).  Output partials are shipped bf16 and summed f32 on host.

v3 changes vs v2:
  - paired score matmuls: the two heads of an m-block live on disjoint
    PE row halves (K=64 each, row groups q0/q64), so their QK^T matmuls
    issue back-to-back and execute CONCURRENTLY on the systolic array.
    Units are (slab, m) pairs (8 units instead of 16), one exp
    ACTIVATE covers both heads' psum banks.
  - xT is kept chunk-major in SBUF (same layout as the host image), so
    every input DMA is a straight contiguous copy with 2-4KB lines.
  - critical first-wave DMAs ride the two hardware-DGE queues
    (sync + scalar); gpsimd's software queue (4us slower to first
    packet) only carries data needed >25us in.
  - warm-up junk matmuls start right after a single ones-memset and
    bridge until the first real data lands, so HAM hits K=8/8 before
    real work begins.
"""

import numpy as np
import ml_dtypes

import concourse.bass as bass
import concourse.mybir as mybir
import concourse.tile as tile
from concourse import bacc
from concourse.bass_utils import run_bass_kernel_spmd

F32 = mybir.dt.float32
BF16 = mybir.dt.bfloat16
AF = mybir.ActivationFunctionType
MULT = mybir.AluOpType.mult

T = 2048  # sequence length
D = 1024  # embed dim
NH = 4  # heads per core
HD = 64  # head dim
DH = NH * HD  # 256, sharded d per core
TT = T // 128  # 16 t tiles
DT = D // 128  # 8 embed tiles
NSLAB = 4  # q slabs of 512
SCALE = 1.0 / np.sqrt(HD)

_NC_CACHE = None


def build():
    nc = bacc.Bacc(None, target_bir_lowering=False, debug=False)

    # inputs are shipped as ready-to-DMA SBUF images (see make_in_maps):
    # xT_img[p, c*4096 + dt*512 + j] = x[c*512+j, dt*128+p]  (chunk-major)
    # wq/wk m-major [p, m*1024 + dt*128 + c]; wv dt-major [p, dt*256 + c];
    # wo i-major [p, i*1024 + c]
    xT_img = nc.declare_dram_parameter("xT", [128, NSLAB * DT * 512], BF16, isOutput=False)
    wq = nc.declare_dram_parameter("Wq", [128, 2 * DT * 128], BF16, isOutput=False)
    wk = nc.declare_dram_parameter("Wk", [128, 2 * DT * 128], BF16, isOutput=False)
    wv = nc.declare_dram_parameter("Wv", [128, DT * DH], BF16, isOutput=False)
    wo = nc.declare_dram_parameter("Wo", [128, 2 * D], BF16, isOutput=False)
    out = nc.declare_dram_parameter("out", [T, D], BF16, isOutput=True)

    with tile.TileContext(nc) as tc:
        with (
            tc.tile_pool(name="persist", bufs=1) as persist,
            tc.tile_pool(name="pt", bufs=2) as pt_pool,
            tc.tile_pool(name="den", bufs=2) as den_pool,
            tc.tile_pool(name="rc", bufs=2) as rc_pool,
            tc.tile_pool(name="stg", bufs=2) as stg_pool,
            tc.tile_pool(name="osb", bufs=2) as osb_pool,
            tc.tile_pool(name="ps_pair", bufs=2, space="PSUM") as ps_pair,
            tc.tile_pool(name="ps_fill", bufs=2, space="PSUM") as ps_fill,
            tc.tile_pool(name="ps_av", bufs=2, space="PSUM") as ps_av,
        ):
            def P(shape, dtype, name):
                return persist.tile(shape, dtype, name=name, tag=name)

            ones_b = P([128, 64], BF16, "ones_b")
            junk = P([128, 16], F32, "junk")
            jout = P([128, 16], F32, "jout")
            jnk_b = P([128, 512], BF16, "jnk_b")

            wq_bf = P([128, DT * DH], BF16, "wq_bf")
            wk_bf = P([128, DT * DH], BF16, "wk_bf")
            wv_bf = P([128, DT * DH], BF16, "wv_bf")
            wo_bf = P([128, 2 * D], BF16, "wo_bf")
            xT = P([128, NSLAB * DT * 512], BF16, "xT")  # chunk-major
            QT = P([128, 2 * T], BF16, "QT")
            KT = P([128, 2 * T], BF16, "KT")
            vbuf = P([128, TT * NH * 65], BF16, "vbuf")
            attnT = P([128, 2 * T], BF16, "attnT")

            # ---- minimal memsets, then warm-up MMs immediately ----
            nc.gpsimd.memset(ones_b[:], 1.0)
            nc.gpsimd.memset(jnk_b[0:1, :], 1.0)
            nc.gpsimd.memset(junk[:], 0.0)
            # exp table prefetch: overlaps the ~2.7us ACT_TABLE_LOAD with
            # the input DMAs instead of paying it at the first real score
            nc.scalar.activation(out=jout[:], in_=junk[:], func=AF.Exp, scale=1.0)

            # HAM warm-up: junk matmuls bridge from ~7.3us until the first
            # real data lands (~13.5us) so real MMs start at 2.4GHz
            warm_ps = ps_fill.tile([128, 512], F32, name="warm", tag="fill")
            for _ in range(14):
                nc.tensor.matmul(
                    warm_ps[0:64, 0:512],
                    lhsT=ones_b[0:1, 0:64],
                    rhs=jnk_b[0:1, 0:512],
                    start=True,
                    stop=True,
                )

            # ---- input DMAs.  sync + scalar are hardware-DGE queues
            # (first packet ~1.4us after issue); gpsimd is software-DGE
            # (first packet ~4-5us after issue) and only carries inputs
            # needed late.  All transfers are contiguous on both sides.
            def xc(c, lo, hi):
                return (c * 4096 + lo * 512, c * 4096 + hi * 512)

            def xdma(eng, c, lo, hi):
                a, b = xc(c, lo, hi)
                eng.dma_start(out=xT[:, a:b], in_=xT_img[:, a:b])

            nc.sync.dma_start(out=wq_bf[:, 0:1024], in_=wq[:, 0:1024])
            nc.scalar.dma_start(out=wk_bf[:, 0:1024], in_=wk[:, 0:1024])
            xdma(nc.sync, 0, 0, 4)
            xdma(nc.scalar, 0, 4, 8)
            nc.sync.dma_start(out=wv_bf[:], in_=wv[:])
            nc.scalar.dma_start(out=wk_bf[:, 1024:2048], in_=wk[:, 1024:2048])
            xdma(nc.sync, 1, 0, 4)
            xdma(nc.scalar, 1, 4, 8)
            # gpsimd: needed from ~25us onward
            nc.gpsimd.dma_start(out=wq_bf[:, 1024:2048], in_=wq[:, 1024:2048])
            xdma(nc.gpsimd, 2, 0, 4)
            xdma(nc.gpsimd, 2, 4, 8)
            xdma(nc.gpsimd, 3, 0, 4)
            xdma(nc.gpsimd, 3, 4, 8)
            nc.gpsimd.dma_start(out=wo_bf[:], in_=wo[:])

            vb3 = vbuf.rearrange("p (t c) -> p t c", c=65)
            nc.gpsimd.memset(vb3[:, :, 64:65], 1.0)
            vb4 = vbuf.rearrange("p (n c) -> p n c", c=65)

            # ---- projection wavefront thunks (PE filler) ----
            def qk_thunk(w_bf, outT, m, c):
                def go():
                    ps = ps_fill.tile([128, 512], F32, name="qk", tag="fill")
                    for dt_ in range(DT):
                        nc.tensor.matmul(
                            ps[:],
                            lhsT=w_bf[
                                :,
                                m * 1024 + dt_ * 128 : m * 1024 + (dt_ + 1) * 128,
                            ],
                            rhs=xT[
                                :, c * 4096 + dt_ * 512 : c * 4096 + (dt_ + 1) * 512
                            ],
                            start=(dt_ == 0),
                            stop=(dt_ == DT - 1),
                        )
                    nc.vector.tensor_copy(
                        outT[:, m * T + c * 512 : m * T + (c + 1) * 512],
                        ps[:],
                    )

                return go

            def qk_thunks(c):
                return [
                    qk_thunk(wq_bf, QT, 0, c),
                    qk_thunk(wk_bf, KT, 0, c),
                    qk_thunk(wq_bf, QT, 1, c),
                    qk_thunk(wk_bf, KT, 1, c),
                ]

            def v_thunks(tts):
                th = []
                for tt in tts:
                    def go(tt=tt):
                        c, r = tt // 4, tt % 4
                        ps = ps_fill.tile([128, 256], F32, name="vp", tag="fill")
                        for dt_ in range(DT):
                            nc.tensor.matmul(
                                ps[:],
                                lhsT=xT[
                                    :,
                                    c * 4096 + dt_ * 512 + r * 128 : c * 4096
                                    + dt_ * 512
                                    + (r + 1) * 128,
                                ],
                                rhs=wv_bf[:, dt_ * DH : (dt_ + 1) * DH],
                                start=(dt_ == 0),
                                stop=(dt_ == DT - 1),
                            )
                        nc.vector.tensor_copy(
                            vb4[:, tt * NH : (tt + 1) * NH, 0:64],
                            ps.rearrange("p (n c) -> p n c", n=NH),
                        )

                    th.append(go)
                return th

            # ---- paired scores: per kt, both heads of the m-block issue
            # back-to-back K=64 matmuls on disjoint row halves (q0 / q64)
            # and run concurrently; one exp ACTIVATE covers both. ----
            def pt_layout(s):
                """pt cols per kt: [A w | B w] compact.  base, width maps."""
                base, width = {}, {}
                b = 0
                for kt in range(4 * (s + 1)):
                    j = kt - 4 * s
                    w = 512 - 128 * j if j > 0 else 512
                    base[kt], width[kt] = b, w
                    b += 2 * w
                return base, width

            def scores_pair_ops(s, m, pt):
                base, width = pt_layout(s)
                ops = []
                for kt in range(4 * (s + 1)):
                    j = kt - 4 * s
                    w = width[kt]
                    pb = base[kt]
                    qoff = 512 - w

                    def go(kt=kt, j=j, w=w, pb=pb, qoff=qoff):
                        ps = ps_pair.tile([128, 1024], F32, name="pssc", tag="pair")
                        for half in range(2):
                            r0 = half * 64
                            nc.tensor.matmul(
                                ps[:, half * 512 : half * 512 + w],
                                lhsT=KT[
                                    r0 : r0 + 64,
                                    m * T + kt * 128 : m * T + (kt + 1) * 128,
                                ],
                                rhs=QT[
                                    r0 : r0 + 64,
                                    m * T + s * 512 + qoff : m * T + (s + 1) * 512,
                                ],
                                start=True,
                                stop=True,
                            )
                        if w == 512:
                            nc.scalar.activation(
                                out=pt[:, pb : pb + 1024],
                                in_=ps[:],
                                func=AF.Exp,
                                scale=float(SCALE),
                            )
                        else:
                            nc.scalar.activation(
                                out=pt[:, pb : pb + 2 * w].rearrange(
                                    "p (h w2) -> p h w2", h=2
                                ),
                                in_=ps.rearrange("p (h c) -> p h c", h=2)[
                                    :, :, 0:w
                                ],
                                func=AF.Exp,
                                scale=float(SCALE),
                            )
                        if j >= 0:
                            # mask the causal triangle at the head of each
                            # head-block (first 128 cols = diag region)
                            for half in range(2):
                                nc.gpsimd.affine_select(
                                    out=pt[:, pb + half * w : pb + half * w + 128],
                                    in_=pt[:, pb + half * w : pb + half * w + 128],
                                    pattern=[[1, 128]],
                                    compare_op=mybir.AluOpType.is_ge,
                                    fill=0.0,
                                    base=0,
                                    channel_multiplier=-1,
                                )

                    # only diagonal tiles (j>=0) need masking; recreate go
                    # accordingly
                    ops.append(go)
                return ops

            # ---- AV + normalize ----
            def av_ops(s, m, hh, pt, ref):
                """AV for head (2m+hh): A-part = off-diag kts, B-part = diag
                kts (gated on last exps + selects)."""
                base, width = pt_layout(s)
                nk = 4 * (s + 1)
                n_h = 2 * m + hh

                def av_a():
                    avb = ps_av.tile([128, 512], F32, name="psav", tag="psav")
                    ref["avb"] = avb
                    for kt in range(4 * s):
                        w = width[kt]
                        nc.tensor.matmul(
                            avb[0:65, 0:512],
                            lhsT=vb4[:, kt * NH + n_h, :],
                            rhs=pt[:, base[kt] + hh * w : base[kt] + hh * w + w],
                            start=(kt == 0),
                            stop=False,
                        )

                def av_b():
                    avb = ref["avb"]
                    for kt in range(4 * s, nk):
                        w = width[kt]
                        o = 512 - w
                        nc.tensor.matmul(
                            avb[0:65, o:512],
                            lhsT=vb4[:, kt * NH + n_h, :],
                            rhs=pt[:, base[kt] + hh * w : base[kt] + hh * w + w],
                            start=(kt == 0),
                            stop=(kt == nk - 1),
                        )

                return av_a, av_b

            def av_cols(s, m, hh, pt, ref, c0, c1, first, last):
                """AV for head (2m+hh) restricted to slab cols [c0, c1)."""
                base, width = pt_layout(s)
                nk = 4 * (s + 1)
                n_h = 2 * m + hh

                def go():
                    if first:
                        avb = ps_av.tile([128, 512], F32, name="psav", tag="psav")
                        ref["avb"] = avb
                    avb = ref["avb"]
                    kts = [kt for kt in range(nk) if 512 - width[kt] < c1]
                    for i_, kt in enumerate(kts):
                        w = width[kt]
                        o = 512 - w
                        lo = max(o, c0)
                        # i_==0 covers the full [c0,c1) range (kt=0 has o=0),
                        # so start=True on it initializes every element
                        nc.tensor.matmul(
                            avb[0:65, lo:c1],
                            lhsT=vb4[:, kt * NH + n_h, :],
                            rhs=pt[
                                :,
                                base[kt] + hh * w + lo - o : base[kt] + hh * w + c1 - o,
                            ],
                            start=(i_ == 0),
                            stop=(last and i_ == len(kts) - 1),
                        )

                return go

            def norm_ops(s, h, ref, q0, q1):
                """den row -> bf16 -> K=1 ones-matmul broadcast -> recip
                -> tensor_tensor multiply -> attnT (DMA shift for odd h)."""
                i_c = h // 2
                c0 = i_c * T + s * 512
                odd = h % 2 == 1
                st = {}

                def d1():
                    den = den_pool.tile([128, 512], BF16, name="den")
                    st["den"] = den
                    nc.vector.tensor_copy(
                        den[64:65, q0:q1], ref["avb"][64:65, q0:q1]
                    )

                def m1():
                    denb = ps_fill.tile([128, 512], F32, name="denb", tag="fill")
                    st["denb"] = denb
                    nc.tensor.matmul(
                        denb[0:64, q0:q1],
                        lhsT=ones_b[64:65, 0:64],
                        rhs=st["den"][64:65, q0:q1],
                        start=True,
                        stop=True,
                    )

                def d2():
                    rc = rc_pool.tile([128, 512], F32, name="rc")
                    st["rc"] = rc
                    nc.vector.reciprocal_approx_fast(
                        rc[0:64, q0:q1], st["denb"][0:64, q0:q1]
                    )

                def d3():
                    if odd:
                        stg = stg_pool.tile([128, 512], BF16, name="stg")
                        st["stg"] = stg
                        dst = stg[0:64, q0:q1]
                    else:
                        dst = attnT[0:64, c0 + q0 : c0 + q1]
                    nc.vector.tensor_tensor(
                        out=dst,
                        in0=ref["avb"][0:64, q0:q1],
                        in1=st["rc"][0:64, q0:q1],
                        op=MULT,
                    )

                def d4():
                    nc.gpsimd.dma_start(
                        out=attnT[64:128, c0 + q0 : c0 + q1],
                        in_=st["stg"][0:64, q0:q1],
                    )

                ops = [d1, m1, d2, d3]
                if odd:
                    ops.append(d4)
                return ops

            # ---- out projection epilogue.  tail=True (final slab): ec0
            # cast moves to the now-idle scalar engine and each 512-col
            # half DMAs out as soon as its cast lands ----
            def epilogue_ops(s, tail=False):
                ops = []
                for tt in range(4 * s, 4 * (s + 1)):
                    st = {}

                    def op_ec(ec, tt=tt, st=st):
                        def go():
                            ps = ps_fill.tile([128, 512], F32, name="opj", tag="fill")
                            st[ec] = ps
                            for i in range(2):
                                nc.tensor.matmul(
                                    ps[:],
                                    lhsT=attnT[
                                        :, i * T + tt * 128 : i * T + (tt + 1) * 128
                                    ],
                                    rhs=wo_bf[
                                        :, i * D + ec * 512 : i * D + (ec + 1) * 512
                                    ],
                                    start=(i == 0),
                                    stop=(i == 1),
                                )

                        return go

                    def cast_dma(ec, tt=tt, st=st):
                        def go():
                            if ec == 0:
                                st["osb"] = osb_pool.tile(
                                    [128, 1024], BF16, name="osb"
                                )
                            if tail and ec == 0:
                                nc.scalar.copy(
                                    st["osb"][:, 0:512], st[ec][:]
                                )
                            else:
                                nc.vector.tensor_copy(
                                    st["osb"][:, ec * 512 : (ec + 1) * 512], st[ec][:]
                                )
                            if tail:
                                eng = nc.sync if ec == 0 else nc.scalar
                                eng.dma_start(
                                    out=out[
                                        tt * 128 : (tt + 1) * 128,
                                        ec * 512 : (ec + 1) * 512,
                                    ],
                                    in_=st["osb"][:, ec * 512 : (ec + 1) * 512],
                                )
                            elif ec == 1:
                                eng = nc.sync if tt % 2 == 0 else nc.gpsimd
                                eng.dma_start(
                                    out=out[tt * 128 : (tt + 1) * 128, :],
                                    in_=st["osb"][:],
                                )

                        return go

                    ops.extend([op_ec(0), op_ec(1), cast_dma(0), cast_dma(1)])
                return ops

            def interleave(a, b):
                if not a:
                    return list(b)
                if not b:
                    return list(a)
                res = []
                nb, na, bi = len(b), len(a), 0
                for i, op in enumerate(a):
                    res.append(op)
                    want = (i + 1) * nb // na
                    while bi < want:
                        res.append(b[bi])
                        bi += 1
                res.extend(b[bi:])
                return res

            # ---- prologue: chunk-0 QK only (v thunks would stall the PE
            # FIFO on the later-arriving wv) ----
            for op in qk_thunks(0):
                op()

            # wave thunks: qk(c+1) consumed at unit (c,0); v tiles for slab
            # c+1 consumed at unit (c,1)
            fills = {
                (0, 0): qk_thunks(1) + v_thunks(range(0, 4)),
                (0, 1): v_thunks(range(4, 8)),
                (1, 0): qk_thunks(2),
                (1, 1): v_thunks(range(8, 12)),
                (2, 0): qk_thunks(3),
                (2, 1): v_thunks(range(12, 16)),
            }

            units = [(s, m) for s in range(NSLAB) for m in range(2)]
            pts = {}
            prev = None
            pending_epi = []
            for idx in range(len(units)):
                s, m = units[idx]
                pts[idx] = pt_pool.tile([128, 2 * TT * 512], BF16, name="pt")
                sc = scores_pair_ops(s, m, pts[idx])
                blist = []
                if prev is not None:
                    ps_, pm_ = units[prev]
                    refA, refB = {}, {}
                    avBa, avBb = av_ops(ps_, pm_, 1, pts[prev], refB)
                    avAa, avAb = av_ops(ps_, pm_, 0, pts[prev], refA)
                    nrmB = norm_ops(ps_, 2 * pm_ + 1, refB, 0, 512)
                    nrmA = norm_ops(ps_, 2 * pm_, refA, 0, 512)
                    epi = list(pending_epi.pop(0)) if pending_epi else []
                    if m == 1 and s >= 1:
                        eall = epilogue_ops(s - 1)
                        epi += eall[:8]
                        pending_epi = [eall[8:16]]
                    fill = fills.get((s, m), [])
                    blist = (
                        fill[:2]
                        + [avBa]
                        + epi[:4]
                        + [avBb]
                        + nrmB
                        + fill[2:]
                        + [avAa, avAb]
                        + nrmA
                        + epi[4:]
                    )
                else:
                    blist = list(fills.get((s, m), []))
                for op in interleave(sc, blist):
                    op()
                prev = idx

            # ---- endgame: unit (3,1) AV/norm interleaved with EP(2)
            # remnant + slab-3 epilogue ----
            s_, m_ = 3, 1
            refA, refB = {}, {}
            avBa, avBb = av_ops(s_, m_, 1, pts[7], refB)
            avA1 = av_cols(s_, m_, 0, pts[7], refA, 0, 256, True, True)
            avA2 = av_cols(s_, m_, 0, pts[7], refA, 256, 512, False, True)
            nrmB = norm_ops(s_, 3, refB, 0, 512)
            nrmA1 = norm_ops(s_, 2, refA, 0, 256)
            nrmA2 = norm_ops(s_, 2, refA, 256, 384)
            nrmA3 = norm_ops(s_, 2, refA, 384, 512)
            eops = epilogue_ops(3, tail=True)
            rem = list(pending_epi.pop(0)) if pending_epi else []

            for op in rem[0:2]:  # EP(2) remnant: exp-free PE filler
                op()
            avBa()
            for op in rem[2:4]:
                op()
            avBb()
            for op in nrmB:
                op()
            avA1()
            for op in rem[4:6]:
                op()
            for op in nrmA1:
                op()
            avA2()
            for op in rem[6:8]:
                op()
            for op in eops[0:4]:  # tt12 (A cols 0:256 + B done)
                op()
            nrmA2[0]()  # den casts + broadcasts first
            nrmA2[1]()
            nrmA3[0]()
            nrmA3[1]()
            for op in eops[4:8]:  # tt13
                op()
            nrmA2[2]()  # recip + multiply chains
            nrmA2[3]()
            nrmA3[2]()
            nrmA3[3]()
            for op in eops[8:12]:  # tt14 (gated on chain A2)
                op()
            for op in eops[12:16]:  # tt15 (gated on chain A3)
                op()

    nc.compile()
    return nc


def _get_nc():
    global _NC_CACHE
    if _NC_CACHE is None:
        _NC_CACHE = build()
    return _NC_CACHE


def make_in_maps(x, Wq, Wk, Wv, Wo):
    bf = ml_dtypes.bfloat16
    x = np.asarray(x, dtype=np.float32)
    WqT = np.asarray(Wq, dtype=np.float32).astype(bf)
    WkT = np.asarray(Wk, dtype=np.float32).astype(bf)
    WvT = np.asarray(Wv, dtype=np.float32).astype(bf)
    WoT = np.asarray(Wo, dtype=np.float32).astype(bf)

    def x_img(xb):  # [1024(d), 2048(t)] -> [128, c*4096 + dt*512 + j]
        return np.ascontiguousarray(
            xb.reshape(DT, 128, NSLAB, 512).transpose(1, 2, 0, 3).reshape(128, -1)
        )

    def qk_img(w):  # [1024, 256] -> m-major [128, m*1024 + dt*128 + c]
        return np.ascontiguousarray(
            w.reshape(DT, 128, 2, 128).transpose(1, 2, 0, 3).reshape(128, -1)
        )

    def v_img(w):  # [1024, 256] -> dt-major [128, dt*256 + c]
        return np.ascontiguousarray(
            w.reshape(DT, 128, DH).transpose(1, 0, 2).reshape(128, -1)
        )

    def o_img(w):  # [256, 1024] -> i-major [128, i*1024 + c]
        return np.ascontiguousarray(
            w.reshape(2, 128, D).transpose(1, 0, 2).reshape(128, -1)
        )

    xTb = [x_img(x[b].T.astype(bf)) for b in range(2)]
    in_maps = []
    for core in range(8):
        b, g = core // 4, core % 4
        sl = slice(g * DH, (g + 1) * DH)
        in_maps.append(
            {
                "xT": xTb[b],
                "Wq": qk_img(WqT[:, sl]),
                "Wk": qk_img(WkT[:, sl]),
                "Wv": v_img(WvT[:, sl]),
                "Wo": o_img(WoT[sl, :]),
            }
        )
    return in_maps


def unshard(results):
    out = np.empty((2, T, D), np.float32)
    for b in range(2):
        acc = results[4 * b]["out"].astype(np.float32)
        for g in range(1, 4):
            acc = acc + results[4 * b + g]["out"].astype(np.float32)
        out[b] = acc
    return out


def kernel(x, Wq, Wk, Wv, Wo):
    nc = _get_nc()
    in_maps = make_in_maps(x, Wq, Wk, Wv, Wo)
    res = run_bass_kernel_spmd(nc, in_maps, core_ids=list(range(8)))
    return unshard(res.results)


# revision 7
# speedup vs baseline: 1.0218x; 1.0218x over previous
"""Distributed causal multi-head attention for 8 TRN2 NeuronCores (v3).

Problem: B=2, T=2048, D=1024, H=16 heads (hd=64), f32 in/out.

Sharding: core i handles batch b=i//4 and head-group g=i%4 (4 heads).
Wq/Wk/Wv column-sharded ([1024, 256] per core), Wo row-sharded
([256, 1024] per core).  Each core computes a partial output projection
for its 4 heads over the full sequence; the host sums the 4 partials
per batch (the unshard step replaces the all-reduce).  Weights and
activations are pre-cast to bf16 on the host; x is laid out transposed
(xT = x^T).  Output partials are shipped bf16 and summed f32 on host.

v3 changes vs v2:
  - paired score matmuls: the two heads of an m-block live on disjoint
    PE row halves (K=64 each, row groups q0/q64), so their QK^T matmuls
    issue back-to-back and execute CONCURRENTLY on the systolic array.
    Units are (slab, m) pairs (8 units instead of 16), one exp
    ACTIVATE covers both heads' psum banks.
  - xT is kept chunk-major in SBUF (same layout as the host image), so
    every input DMA is a straight contiguous copy with 2-4KB lines.
  - critical first-wave DMAs ride the two hardware-DGE queues
    (sync + scalar); gpsimd's software queue (4us slower to first
    packet) only carries data needed >25us in.
  - warm-up junk matmuls start right after a single ones-memset and
    bridge until the first real data lands, so HAM hits K=8/8 before
    real work begins.
"""

import numpy as np
import ml_dtypes

import concourse.bass as bass
import concourse.mybir as mybir
import concourse.tile as tile
from concourse import bacc
from concourse.bass_utils import run_bass_kernel_spmd

F32 = mybir.dt.float32
BF16 = mybir.dt.bfloat16
AF = mybir.ActivationFunctionType
MULT = mybir.AluOpType.mult

T = 2048  # sequence length
D = 1024  # embed dim
NH = 4  # heads per core
HD = 64  # head dim
DH = NH * HD  # 256, sharded d per core
TT = T // 128  # 16 t tiles
DT = D // 128  # 8 embed tiles
NSLAB = 4  # q slabs of 512
SCALE = 1.0 / np.sqrt(HD)

_NC_CACHE = None


def build():
    nc = bacc.Bacc(None, target_bir_lowering=False, debug=False)

    # inputs are shipped as ready-to-DMA SBUF images (see make_in_maps):
    # xT_img[p, c*4096 + dt*512 + j] = x[c*512+j, dt*128+p]  (chunk-major)
    # wq/wk m-major [p, m*1024 + dt*128 + c]; wv dt-major [p, dt*256 + c];
    # wo i-major [p, i*1024 + c]
    xT_img = nc.declare_dram_parameter("xT", [128, NSLAB * DT * 512], BF16, isOutput=False)
    wq = nc.declare_dram_parameter("Wq", [128, 2 * DT * 128], BF16, isOutput=False)
    wk = nc.declare_dram_parameter("Wk", [128, 2 * DT * 128], BF16, isOutput=False)
    wv = nc.declare_dram_parameter("Wv", [128, DT * DH], BF16, isOutput=False)
    wo = nc.declare_dram_parameter("Wo", [128, 2 * D], BF16, isOutput=False)
    out = nc.declare_dram_parameter("out", [T, D], BF16, isOutput=True)

    with tile.TileContext(nc) as tc:
        with (
            tc.tile_pool(name="persist", bufs=1) as persist,
            tc.tile_pool(name="pt", bufs=3) as pt_pool,
            tc.tile_pool(name="den", bufs=2) as den_pool,
            tc.tile_pool(name="rc", bufs=2) as rc_pool,
            tc.tile_pool(name="stg", bufs=2) as stg_pool,
            tc.tile_pool(name="osb", bufs=2) as osb_pool,
            tc.tile_pool(name="ps_pair", bufs=2, space="PSUM") as ps_pair,
            tc.tile_pool(name="ps_fill", bufs=2, space="PSUM") as ps_fill,
            tc.tile_pool(name="ps_av", bufs=2, space="PSUM") as ps_av,
        ):
            def P(shape, dtype, name):
                return persist.tile(shape, dtype, name=name, tag=name)

            ones_b = P([128, 64], BF16, "ones_b")
            junk = P([128, 16], F32, "junk")
            jout = P([128, 16], F32, "jout")
            jnk_b = P([128, 512], BF16, "jnk_b")

            wq_bf = P([128, DT * DH], BF16, "wq_bf")
            wk_bf = P([128, DT * DH], BF16, "wk_bf")
            wv_bf = P([128, DT * DH], BF16, "wv_bf")
            wo_bf = P([128, 2 * D], BF16, "wo_bf")
            xT = P([128, NSLAB * DT * 512], BF16, "xT")  # chunk-major
            QT = P([128, 2 * T], BF16, "QT")
            KT = P([128, 2 * T], BF16, "KT")
            vbuf = P([128, TT * NH * 65], BF16, "vbuf")
            attnT = P([128, 2 * T], BF16, "attnT")

            # ---- minimal memsets, then warm-up MMs immediately ----
            nc.gpsimd.memset(ones_b[:], 1.0)
            nc.gpsimd.memset(jnk_b[0:1, :], 1.0)
            nc.gpsimd.memset(junk[:], 0.0)
            # exp table prefetch: overlaps the ~2.7us ACT_TABLE_LOAD with
            # the input DMAs instead of paying it at the first real score
            nc.scalar.activation(out=jout[:], in_=junk[:], func=AF.Exp, scale=1.0)

            # HAM warm-up: junk matmuls bridge from ~7.3us until the first
            # real data lands (~12us) so real MMs start at 2.4GHz
            warm_ps = ps_fill.tile([128, 512], F32, name="warm", tag="fill")
            for _ in range(10):
                nc.tensor.matmul(
                    warm_ps[0:64, 0:512],
                    lhsT=ones_b[0:1, 0:64],
                    rhs=jnk_b[0:1, 0:512],
                    start=True,
                    stop=True,
                )

            # ---- input DMAs.  sync + scalar are hardware-DGE queues
            # (first packet ~1.4us after issue); gpsimd is software-DGE
            # (first packet ~4-5us after issue) and only carries inputs
            # needed late.  All transfers are contiguous on both sides.
            # chunk 0 is split per-2dt so the first qk thunk's matmuls can
            # stream as slices land instead of waiting for the full 1MB.
            def xc(c, lo, hi):
                return (c * 4096 + lo * 512, c * 4096 + hi * 512)

            def xdma(eng, c, lo, hi):
                a, b = xc(c, lo, hi)
                eng.dma_start(out=xT[:, a:b], in_=xT_img[:, a:b])

            nc.sync.dma_start(out=wq_bf[:, 0:1024], in_=wq[:, 0:1024])
            nc.scalar.dma_start(out=wk_bf[:, 0:1024], in_=wk[:, 0:1024])
            xdma(nc.sync, 0, 0, 2)
            xdma(nc.scalar, 0, 2, 4)
            xdma(nc.sync, 0, 4, 6)
            xdma(nc.scalar, 0, 6, 8)
            nc.sync.dma_start(out=wv_bf[:], in_=wv[:])
            nc.scalar.dma_start(out=wk_bf[:, 1024:2048], in_=wk[:, 1024:2048])
            xdma(nc.sync, 1, 0, 4)
            xdma(nc.scalar, 1, 4, 8)
            # gpsimd: needed from ~25us onward
            nc.gpsimd.dma_start(out=wq_bf[:, 1024:2048], in_=wq[:, 1024:2048])
            xdma(nc.gpsimd, 2, 0, 4)
            xdma(nc.gpsimd, 2, 4, 8)
            xdma(nc.gpsimd, 3, 0, 4)
            xdma(nc.gpsimd, 3, 4, 8)
            nc.gpsimd.dma_start(out=wo_bf[:], in_=wo[:])

            vb3 = vbuf.rearrange("p (t c) -> p t c", c=65)
            nc.gpsimd.memset(vb3[:, :, 64:65], 1.0)
            vb4 = vbuf.rearrange("p (n c) -> p n c", c=65)

            # ---- projection wavefront thunks (PE filler) ----
            def qk_thunk(w_bf, outT, m, c):
                def go():
                    ps = ps_fill.tile([128, 512], F32, name="qk", tag="fill")
                    for dt_ in range(DT):
                        nc.tensor.matmul(
                            ps[:],
                            lhsT=w_bf[
                                :,
                                m * 1024 + dt_ * 128 : m * 1024 + (dt_ + 1) * 128,
                            ],
                            rhs=xT[
                                :, c * 4096 + dt_ * 512 : c * 4096 + (dt_ + 1) * 512
                            ],
                            start=(dt_ == 0),
                            stop=(dt_ == DT - 1),
                        )
                    nc.vector.tensor_copy(
                        outT[:, m * T + c * 512 : m * T + (c + 1) * 512],
                        ps[:],
                    )

                return go

            def qk_thunks(c):
                return [
                    qk_thunk(wq_bf, QT, 0, c),
                    qk_thunk(wk_bf, KT, 0, c),
                    qk_thunk(wq_bf, QT, 1, c),
                    qk_thunk(wk_bf, KT, 1, c),
                ]

            def v_thunks(tts):
                th = []
                for tt in tts:
                    def go(tt=tt):
                        c, r = tt // 4, tt % 4
                        ps = ps_fill.tile([128, 256], F32, name="vp", tag="fill")
                        for dt_ in range(DT):
                            nc.tensor.matmul(
                                ps[:],
                                lhsT=xT[
                                    :,
                                    c * 4096 + dt_ * 512 + r * 128 : c * 4096
                                    + dt_ * 512
                                    + (r + 1) * 128,
                                ],
                                rhs=wv_bf[:, dt_ * DH : (dt_ + 1) * DH],
                                start=(dt_ == 0),
                                stop=(dt_ == DT - 1),
                            )
                        nc.vector.tensor_copy(
                            vb4[:, tt * NH : (tt + 1) * NH, 0:64],
                            ps.rearrange("p (n c) -> p n c", n=NH),
                        )

                    th.append(go)
                return th

            # ---- paired scores: per kt, both heads of the m-block issue
            # back-to-back K=64 matmuls on disjoint row halves (q0 / q64)
            # and run concurrently; one exp ACTIVATE covers both. ----
            def pt_layout(s):
                """pt cols per kt: [A w | B w] compact.  base, width maps."""
                base, width = {}, {}
                b = 0
                for kt in range(4 * (s + 1)):
                    j = kt - 4 * s
                    w = 512 - 128 * j if j > 0 else 512
                    base[kt], width[kt] = b, w
                    b += 2 * w
                return base, width

            def scores_pair_ops(s, m, pt):
                base, width = pt_layout(s)
                ops = []
                for kt in range(4 * (s + 1)):
                    j = kt - 4 * s
                    w = width[kt]
                    pb = base[kt]
                    qoff = 512 - w

                    def go(kt=kt, j=j, w=w, pb=pb, qoff=qoff):
                        ps = ps_pair.tile([128, 1024], F32, name="pssc", tag="pair")
                        for half in range(2):
                            r0 = half * 64
                            nc.tensor.matmul(
                                ps[:, half * 512 : half * 512 + w],
                                lhsT=KT[
                                    r0 : r0 + 64,
                                    m * T + kt * 128 : m * T + (kt + 1) * 128,
                                ],
                                rhs=QT[
                                    r0 : r0 + 64,
                                    m * T + s * 512 + qoff : m * T + (s + 1) * 512,
                                ],
                                start=True,
                                stop=True,
                            )
                        if w == 512:
                            nc.scalar.activation(
                                out=pt[:, pb : pb + 1024],
                                in_=ps[:],
                                func=AF.Exp,
                                scale=float(SCALE),
                            )
                        else:
                            nc.scalar.activation(
                                out=pt[:, pb : pb + 2 * w].rearrange(
                                    "p (h w2) -> p h w2", h=2
                                ),
                                in_=ps.rearrange("p (h c) -> p h c", h=2)[
                                    :, :, 0:w
                                ],
                                func=AF.Exp,
                                scale=float(SCALE),
                            )
                        if j >= 0:
                            # mask the causal triangle at the head of each
                            # head-block (first 128 cols = diag region)
                            for half in range(2):
                                nc.gpsimd.affine_select(
                                    out=pt[:, pb + half * w : pb + half * w + 128],
                                    in_=pt[:, pb + half * w : pb + half * w + 128],
                                    pattern=[[1, 128]],
                                    compare_op=mybir.AluOpType.is_ge,
                                    fill=0.0,
                                    base=0,
                                    channel_multiplier=-1,
                                )

                    # only diagonal tiles (j>=0) need masking; recreate go
                    # accordingly
                    ops.append(go)
                return ops

            # ---- AV + normalize ----
            def av_ops(s, m, hh, pt, ref):
                """AV for head (2m+hh): A-part = off-diag kts, B-part = diag
                kts (gated on last exps + selects)."""
                base, width = pt_layout(s)
                nk = 4 * (s + 1)
                n_h = 2 * m + hh

                def av_a():
                    avb = ps_av.tile([128, 512], F32, name="psav", tag="psav")
                    ref["avb"] = avb
                    for kt in range(4 * s):
                        w = width[kt]
                        nc.tensor.matmul(
                            avb[0:65, 0:512],
                            lhsT=vb4[:, kt * NH + n_h, :],
                            rhs=pt[:, base[kt] + hh * w : base[kt] + hh * w + w],
                            start=(kt == 0),
                            stop=False,
                        )

                def av_b():
                    avb = ref["avb"]
                    for kt in range(4 * s, nk):
                        w = width[kt]
                        o = 512 - w
                        nc.tensor.matmul(
                            avb[0:65, o:512],
                            lhsT=vb4[:, kt * NH + n_h, :],
                            rhs=pt[:, base[kt] + hh * w : base[kt] + hh * w + w],
                            start=(kt == 0),
                            stop=(kt == nk - 1),
                        )

                return av_a, av_b

            def av_cols(s, m, hh, pt, ref, c0, c1, first, last):
                """AV for head (2m+hh) restricted to slab cols [c0, c1)."""
                base, width = pt_layout(s)
                nk = 4 * (s + 1)
                n_h = 2 * m + hh

                def go():
                    if first:
                        avb = ps_av.tile([128, 512], F32, name="psav", tag="psav")
                        ref["avb"] = avb
                    avb = ref["avb"]
                    kts = [kt for kt in range(nk) if 512 - width[kt] < c1]
                    for i_, kt in enumerate(kts):
                        w = width[kt]
                        o = 512 - w
                        lo = max(o, c0)
                        # i_==0 covers the full [c0,c1) range (kt=0 has o=0),
                        # so start=True on it initializes every element
                        nc.tensor.matmul(
                            avb[0:65, lo:c1],
                            lhsT=vb4[:, kt * NH + n_h, :],
                            rhs=pt[
                                :,
                                base[kt] + hh * w + lo - o : base[kt] + hh * w + c1 - o,
                            ],
                            start=(i_ == 0),
                            stop=(last and i_ == len(kts) - 1),
                        )

                return go

            def norm_ops(s, h, ref, q0, q1):
                """den row -> bf16 -> K=1 ones-matmul broadcast -> recip
                -> tensor_tensor multiply -> attnT (DMA shift for odd h)."""
                i_c = h // 2
                c0 = i_c * T + s * 512
                odd = h % 2 == 1
                st = {}

                def d1():
                    den = den_pool.tile([128, 512], BF16, name="den")
                    st["den"] = den
                    nc.vector.tensor_copy(
                        den[64:65, q0:q1], ref["avb"][64:65, q0:q1]
                    )

                def m1():
                    denb = ps_fill.tile([128, 512], F32, name="denb", tag="fill")
                    st["denb"] = denb
                    nc.tensor.matmul(
                        denb[0:64, q0:q1],
                        lhsT=ones_b[64:65, 0:64],
                        rhs=st["den"][64:65, q0:q1],
                        start=True,
                        stop=True,
                    )

                def d2():
                    rc = rc_pool.tile([128, 512], F32, name="rc")
                    st["rc"] = rc
                    nc.vector.reciprocal_approx_fast(
                        rc[0:64, q0:q1], st["denb"][0:64, q0:q1]
                    )

                def d3():
                    if odd:
                        stg = stg_pool.tile([128, 512], BF16, name="stg")
                        st["stg"] = stg
                        dst = stg[0:64, q0:q1]
                    else:
                        dst = attnT[0:64, c0 + q0 : c0 + q1]
                    nc.vector.tensor_tensor(
                        out=dst,
                        in0=ref["avb"][0:64, q0:q1],
                        in1=st["rc"][0:64, q0:q1],
                        op=MULT,
                    )

                def d4():
                    nc.gpsimd.dma_start(
                        out=attnT[64:128, c0 + q0 : c0 + q1],
                        in_=st["stg"][0:64, q0:q1],
                    )

                ops = [d1, m1, d2, d3]
                if odd:
                    ops.append(d4)
                return ops

            # ---- out projection epilogue.  tail=True (final slab): ec0
            # cast moves to the now-idle scalar engine and each 512-col
            # half DMAs out as soon as its cast lands ----
            def epilogue_ops(s, tail=False):
                ops = []
                for tt in range(4 * s, 4 * (s + 1)):
                    st = {}

                    def op_ec(ec, tt=tt, st=st):
                        def go():
                            ps = ps_fill.tile([128, 512], F32, name="opj", tag="fill")
                            st[ec] = ps
                            for i in range(2):
                                nc.tensor.matmul(
                                    ps[:],
                                    lhsT=attnT[
                                        :, i * T + tt * 128 : i * T + (tt + 1) * 128
                                    ],
                                    rhs=wo_bf[
                                        :, i * D + ec * 512 : i * D + (ec + 1) * 512
                                    ],
                                    start=(i == 0),
                                    stop=(i == 1),
                                )

                        return go

                    def cast_dma(ec, tt=tt, st=st):
                        def go():
                            if ec == 0:
                                st["osb"] = osb_pool.tile(
                                    [128, 1024], BF16, name="osb"
                                )
                            if tail and ec == 0:
                                nc.scalar.copy(
                                    st["osb"][:, 0:512], st[ec][:]
                                )
                            else:
                                nc.vector.tensor_copy(
                                    st["osb"][:, ec * 512 : (ec + 1) * 512], st[ec][:]
                                )
                            if tail:
                                eng = nc.sync if ec == 0 else nc.scalar
                                eng.dma_start(
                                    out=out[
                                        tt * 128 : (tt + 1) * 128,
                                        ec * 512 : (ec + 1) * 512,
                                    ],
                                    in_=st["osb"][:, ec * 512 : (ec + 1) * 512],
                                )
                            elif ec == 1:
                                eng = nc.sync if tt % 2 == 0 else nc.gpsimd
                                eng.dma_start(
                                    out=out[tt * 128 : (tt + 1) * 128, :],
                                    in_=st["osb"][:],
                                )

                        return go

                    ops.extend([op_ec(0), op_ec(1), cast_dma(0), cast_dma(1)])
                return ops

            def interleave(a, b):
                if not a:
                    return list(b)
                if not b:
                    return list(a)
                res = []
                nb, na, bi = len(b), len(a), 0
                for i, op in enumerate(a):
                    res.append(op)
                    want = (i + 1) * nb // na
                    while bi < want:
                        res.append(b[bi])
                        bi += 1
                res.extend(b[bi:])
                return res

            # ---- prologue: chunk-0 QK only (v thunks would stall the PE
            # FIFO on the later-arriving wv) ----
            for op in qk_thunks(0):
                op()

            # wave thunks: qk(c+1) consumed at unit (c,0); v tiles for slab
            # c+1 consumed at unit (c,1)
            fills = {
                (0, 0): qk_thunks(1) + v_thunks(range(0, 4)),
                (0, 1): v_thunks(range(4, 8)),
                (1, 0): qk_thunks(2),
                (1, 1): v_thunks(range(8, 12)),
                (2, 0): qk_thunks(3),
                (2, 1): v_thunks(range(12, 16)),
            }

            units = [(s, m) for s in range(NSLAB) for m in range(2)]
            # hoist[i] = how many of unit i+1's (off-diag) score ops are
            # emitted during unit i.  Pulls slab-2/3 exps earlier so the
            # scalar engine isn't the critical path in the last units.
            hoist = {4: 4, 5: 8, 6: 8}
            pts = {}
            scs = {}
            prev = None
            pending_epi = []
            for idx in range(len(units)):
                s, m = units[idx]
                if idx not in pts:
                    pts[idx] = pt_pool.tile([128, 2 * TT * 512], BF16, name="pt")
                    scs[idx] = scores_pair_ops(s, m, pts[idx])
                taken = hoist.get(idx - 1, 0)
                sc = scs[idx][taken:]
                h = hoist.get(idx, 0)
                if h:
                    s2, m2 = units[idx + 1]
                    pts[idx + 1] = pt_pool.tile(
                        [128, 2 * TT * 512], BF16, name="pt"
                    )
                    scs[idx + 1] = scores_pair_ops(s2, m2, pts[idx + 1])
                    sc = sc + scs[idx + 1][:h]
                blist = []
                if prev is not None:
                    ps_, pm_ = units[prev]
                    refA, refB = {}, {}
                    avBa, avBb = av_ops(ps_, pm_, 1, pts[prev], refB)
                    avAa, avAb = av_ops(ps_, pm_, 0, pts[prev], refA)
                    nrmB = norm_ops(ps_, 2 * pm_ + 1, refB, 0, 512)
                    nrmA = norm_ops(ps_, 2 * pm_, refA, 0, 512)
                    epi = list(pending_epi.pop(0)) if pending_epi else []
                    if m == 1 and s >= 1:
                        eall = epilogue_ops(s - 1)
                        epi += eall[:8]
                        pending_epi = [eall[8:16]]
                    fill = fills.get((s, m), [])
                    blist = (
                        fill[:2]
                        + [avBa]
                        + epi[:4]
                        + [avBb]
                        + nrmB
                        + fill[2:]
                        + [avAa, avAb]
                        + nrmA
                        + epi[4:]
                    )
                else:
                    blist = list(fills.get((s, m), []))
                for op in interleave(sc, blist):
                    op()
                prev = idx

            # ---- endgame: unit (3,1) AV/norm interleaved with EP(2)
            # remnant + slab-3 epilogue ----
            s_, m_ = 3, 1
            refA, refB = {}, {}
            avBa, avBb = av_ops(s_, m_, 1, pts[7], refB)
            avA1 = av_cols(s_, m_, 0, pts[7], refA, 0, 256, True, True)
            avA2 = av_cols(s_, m_, 0, pts[7], refA, 256, 512, False, True)
            nrmB = norm_ops(s_, 3, refB, 0, 512)
            nrmA1 = norm_ops(s_, 2, refA, 0, 256)
            nrmA2 = norm_ops(s_, 2, refA, 256, 384)
            nrmA3 = norm_ops(s_, 2, refA, 384, 512)
            eops = epilogue_ops(3, tail=True)
            rem = list(pending_epi.pop(0)) if pending_epi else []

            for op in rem[0:2]:  # EP(2) remnant: exp-free PE filler
                op()
            avBa()
            for op in rem[2:4]:
                op()
            avBb()
            for op in nrmB:
                op()
            avA1()
            for op in rem[4:6]:
                op()
            for op in nrmA1:
                op()
            avA2()
            for op in rem[6:8]:
                op()
            for op in eops[0:4]:  # tt12 (A cols 0:256 + B done)
                op()
            nrmA2[0]()  # den casts + broadcasts first
            nrmA2[1]()
            nrmA3[0]()
            nrmA3[1]()
            for op in eops[4:8]:  # tt13
                op()
            nrmA2[2]()  # recip + multiply chains
            nrmA2[3]()
            nrmA3[2]()
            nrmA3[3]()
            for op in eops[8:12]:  # tt14 (gated on chain A2)
                op()
            for op in eops[12:16]:  # tt15 (gated on chain A3)
                op()

    nc.compile()
    return nc


def _get_nc():
    global _NC_CACHE
    if _NC_CACHE is None:
        _NC_CACHE = build()
    return _NC_CACHE


def make_in_maps(x, Wq, Wk, Wv, Wo):
    bf = ml_dtypes.bfloat16
    x = np.asarray(x, dtype=np.float32)
    WqT = np.asarray(Wq, dtype=np.float32).astype(bf)
    WkT = np.asarray(Wk, dtype=np.float32).astype(bf)
    WvT = np.asarray(Wv, dtype=np.float32).astype(bf)
    WoT = np.asarray(Wo, dtype=np.float32).astype(bf)

    def x_img(xb):  # [1024(d), 2048(t)] -> [128, c*4096 + dt*512 + j]
        return np.ascontiguousarray(
            xb.reshape(DT, 128, NSLAB, 512).transpose(1, 2, 0, 3).reshape(128, -1)
        )

    def qk_img(w):  # [1024, 256] -> m-major [128, m*1024 + dt*128 + c]
        return np.ascontiguousarray(
            w.reshape(DT, 128, 2, 128).transpose(1, 2, 0, 3).reshape(128, -1)
        )

    def v_img(w):  # [1024, 256] -> dt-major [128, dt*256 + c]
        return np.ascontiguousarray(
            w.reshape(DT, 128, DH).transpose(1, 0, 2).reshape(128, -1)
        )

    def o_img(w):  # [256, 1024] -> i-major [128, i*1024 + c]
        return np.ascontiguousarray(
            w.reshape(2, 128, D).transpose(1, 0, 2).reshape(128, -1)
        )

    xTb = [x_img(x[b].T.astype(bf)) for b in range(2)]
    in_maps = []
    for core in range(8):
        b, g = core // 4, core % 4
        sl = slice(g * DH, (g + 1) * DH)
        in_maps.append(
            {
                "xT": xTb[b],
                "Wq": qk_img(WqT[:, sl]),
                "Wk": qk_img(WkT[:, sl]),
                "Wv": v_img(WvT[:, sl]),
                "Wo": o_img(WoT[sl, :]),
            }
        )
    return in_maps


def unshard(results):
    out = np.empty((2, T, D), np.float32)
    for b in range(2):
        acc = results[4 * b]["out"].astype(np.float32)
        for g in range(1, 4):
            acc = acc + results[4 * b + g]["out"].astype(np.float32)
        out[b] = acc
    return out


def kernel(x, Wq, Wk, Wv, Wo):
    nc = _get_nc()
    in_maps = make_in_maps(x, Wq, Wk, Wv, Wo)
    res = run_bass_kernel_spmd(nc, in_maps, core_ids=list(range(8)))
    return unshard(res.results)
